# revision 2
# baseline (speedup 1.0000x reference)
"""MoE top-2 routed FFN (B=4, S=2048, D=1024, H=2048, E=8) on 8 TRN2 NeuronCores.

Strategy (expert-parallel, matching the sharding hint):
  - Host computes the tiny gate (softmax top-2) and builds per-expert token
    lists ("all-to-all dispatch" done at the sharding step).
  - Core e receives the tokens routed to expert e (gathered, transposed,
    zero-padded to capacity C), plus expert e's weights pre-packed into the
    exact tile layouts the kernel consumes.
  - Each core runs a dense FFN  out = coef * ((relu(x@W1.T)^2 * (x@W3.T)) @ W2.T)
    over its C tokens.  All matmuls run in bf16 with fp32 PSUM accumulation;
    output is stored bf16 (measured end-to-end rel err ~6e-3 vs 2e-2 budget),
    halving output DMA traffic.
  - Host scatter-adds the per-expert outputs back ("combine").

Per-core kernel structure (group-outer passes; weights streamed per pass):
  pass0: cols [0,512)        - starts as soon as w1[m0] + x-head tiles land;
                               no PE warmup block: the first real matmuls warm
                               the HAM clock gate themselves.
  pass1: cols [512,1536)     - two 512-col groups share each k-loop so bass
  pass2: cols [1536,2176)      emits one LDWEIGHTS per (m,k) for both groups.
  Phase 2 (out tiles) is emitted between/after passes so its PSUM->DVE->DMA
  drain and the output DMAs spread across the whole kernel span instead of
  bunching in a tail.
"""

import os
import sys

import numpy as np

if os.path.isdir("/opt/trn_rl_repo") and "/opt/trn_rl_repo" not in sys.path:
    sys.path.insert(0, "/opt/trn_rl_repo")

import ml_dtypes

import concourse.bacc as bacc
import concourse.mybir as mybir
from concourse.bass_utils import run_bass_kernel_spmd
from concourse.tile import TileContext

B, S, D, H, E = 4, 2048, 1024, 2048, 8
N = B * S
P = 128
KT = D // P   # 8 contraction tiles over D
MT = H // P   # 16 tiles over H
HEAD = 512    # x columns resident before the first matmul

F32 = mybir.dt.float32
BF16 = mybir.dt.bfloat16
BF16_NP = ml_dtypes.bfloat16

# Set by test harness to capture profiling info.
TRACE = False
LAST_RESULTS = None


def build_kernel(C):
    TT = C // P
    TAIL = C - HEAD
    nc = bacc.Bacc("TRN2", target_bir_lowering=False)

    xh_d = nc.dram_tensor("xh", [KT, P, HEAD], BF16, kind="ExternalInput")
    xtl_d = nc.dram_tensor("xtl", [KT, P, TAIL], BF16, kind="ExternalInput")
    w1p = nc.dram_tensor("w1p", [MT, P, KT * P], BF16, kind="ExternalInput")
    w3p = nc.dram_tensor("w3p", [MT, P, KT * P], BF16, kind="ExternalInput")
    w2p = nc.dram_tensor("w2p", [MT, P, D], BF16, kind="ExternalInput")
    cf = nc.dram_tensor("cf", [P, TT], F32, kind="ExternalInput")
    out = nc.dram_tensor("out", [TT, 2, P, 512], BF16, kind="ExternalOutput")

    # token-column groups per pass; groups within a pass share each k-loop
    passes = [
        [(0, HEAD)],
        [(HEAD, 512), (HEAD + 512, 512)],
        [(HEAD + 1024, 512), (HEAD + 1536, C - HEAD - 1536)],
    ]
    assert C - HEAD - 1536 in range(1, 513)

    with TileContext(nc) as tc:
        with (
            tc.tile_pool(name="xh_pool", bufs=1) as xh_pool,
            tc.tile_pool(name="xtl_pool", bufs=1) as xtl_pool,
            tc.tile_pool(name="g_pool", bufs=1) as g_pool,
            tc.tile_pool(name="w13_pool", bufs=4) as w13_pool,
            tc.tile_pool(name="w2_pool", bufs=MT) as w2_pool,
            tc.tile_pool(name="tmp_pool", bufs=2) as tmp_pool,
            tc.tile_pool(name="ob_pool", bufs=4) as ob_pool,
            tc.tile_pool(name="const_pool", bufs=1) as const_pool,
            tc.tile_pool(name="psAB", bufs=3, space="PSUM") as psAB_pool,
            tc.tile_pool(name="psO", bufs=2, space="PSUM") as psO_pool,
        ):
            # x head: needed by the very first matmuls
            xhs = []
            for k in range(KT):
                t = xh_pool.tile([P, HEAD], BF16, tag=f"xh{k}", name=f"xh_{k}")
                nc.sync.dma_start(t[:], xh_d[k])
                xhs.append(t)
            xtls = [None] * KT

            def emit_xtl(ks):
                for k in ks:
                    t = xtl_pool.tile([P, TAIL], BF16, tag=f"xl{k}",
                                      name=f"xtl_{k}")
                    nc.sync.dma_start(t[:], xtl_d[k])
                    xtls[k] = t

            def xslice(k, g0, gw):
                if g0 < HEAD:
                    assert g0 + gw <= HEAD
                    return xhs[k][:, g0:g0 + gw]
                return xtls[k][:, g0 - HEAD:g0 - HEAD + gw]

            gts = []
            for m in range(MT):
                gts.append(g_pool.tile([P, C], BF16, tag=f"g{m}",
                                       name=f"g_{m}"))

            def phase1_pass(pi, groups):
                for m in range(MT):
                    w1t = w13_pool.tile([P, KT * P], BF16, tag="w1t",
                                        name=f"w1_{pi}_{m}")
                    nc.sync.dma_start(w1t[:], w1p[m])
                    w3t = w13_pool.tile([P, KT * P], BF16, tag="w3t",
                                        name=f"w3_{pi}_{m}")
                    nc.sync.dma_start(w3t[:], w3p[m])
                    if pi == 0 and m == 5:
                        emit_xtl(range(0, 4))
                    if pi == 0 and m == 9:
                        emit_xtl(range(4, KT))
                    psAs, psBs = [], []
                    for gi, (g0, gw) in enumerate(groups):
                        psAs.append(psAB_pool.tile(
                            [P, 512], F32, tag="psA", name=f"psA{pi}_{m}_{gi}"))
                        psBs.append(psAB_pool.tile(
                            [P, 512], F32, tag="psB", name=f"psB{pi}_{m}_{gi}"))
                    # shared k-loop: one LDWEIGHTS per (m,k) serves all groups
                    for k in range(KT):
                        for (g0, gw), ps in zip(groups, psAs):
                            nc.tensor.matmul(
                                ps[:, :gw],
                                w1t[:, k * P:(k + 1) * P],
                                xslice(k, g0, gw),
                                start=(k == 0), stop=(k == KT - 1),
                            )
                    for k in range(KT):
                        for (g0, gw), ps in zip(groups, psBs):
                            nc.tensor.matmul(
                                ps[:, :gw],
                                w3t[:, k * P:(k + 1) * P],
                                xslice(k, g0, gw),
                                start=(k == 0), stop=(k == KT - 1),
                            )
                    for (g0, gw), psA, psB in zip(groups, psAs, psBs):
                        r = tmp_pool.tile([P, 512], BF16, tag="r",
                                          name=f"r{pi}_{m}_{g0}")
                        nc.vector.tensor_relu(r[:, :gw], psA[:, :gw])
                        t2 = tmp_pool.tile([P, 512], BF16, tag="t2",
                                           name=f"t2{pi}_{m}_{g0}")
                        nc.vector.tensor_mul(t2[:, :gw], r[:, :gw], r[:, :gw])
                        nc.vector.tensor_mul(
                            gts[m][:, g0:g0 + gw], t2[:, :gw], psB[:, :gw])

            w2ts = []

            def emit_w2_cf():
                cft = const_pool.tile([P, TT], F32, tag="cft")
                nc.sync.dma_start(cft[:], cf[:])
                for hk in range(MT):
                    w2t = w2_pool.tile([P, D], BF16, tag="w2t",
                                       name=f"w2_{hk}")
                    nc.sync.dma_start(w2t[:], w2p[hk])
                    w2ts.append(w2t)
                return cft

            def phase2_unit(t, dg, cft):
                pso = psO_pool.tile([P, 512], F32, tag="psO",
                                    name=f"psO_{t}_{dg}")
                for hk in range(MT):
                    nc.tensor.matmul(
                        pso[:],
                        gts[hk][:, t * P:(t + 1) * P],
                        w2ts[hk][:, dg * 512:(dg + 1) * 512],
                        start=(hk == 0), stop=(hk == MT - 1),
                    )
                ob = ob_pool.tile([P, 512], BF16, tag="ob",
                                  name=f"ob_{t}_{dg}")
                nc.vector.tensor_scalar_mul(ob[:], pso[:], cft[:, t:t + 1])
                nc.sync.dma_start(out[t, dg], ob[:])

            phase1_pass(0, passes[0])
            phase1_pass(1, passes[1])
            cft = emit_w2_cf()
            # tiles fully produced after pass1: cols [0, 1536) = t0..t11
            for t in range(12):
                for dg in range(2):
                    phase2_unit(t, dg, cft)
            phase1_pass(2, passes[2])
            for t in range(12, TT):
                for dg in range(2):
                    phase2_unit(t, dg, cft)

    if not nc.is_finalized():
        nc.finalize()
    return nc


def kernel(x, W1, W2, W3, gate_w, gate_b):
    global LAST_RESULTS

    xf = np.ascontiguousarray(x.reshape(N, D).astype(np.float32, copy=False))

    # ---- gate: softmax + top-2 (tiny, done on host) ------------------------
    logits = xf @ gate_w.T.astype(np.float32) + gate_b.astype(np.float32)
    logits -= logits.max(axis=-1, keepdims=True)
    probs = np.exp(logits)
    probs /= probs.sum(axis=-1, keepdims=True)
    order = np.argsort(-probs, axis=-1, kind="stable")
    i1, i2 = order[:, 0], order[:, 1]
    ar = np.arange(N)
    p1, p2 = probs[ar, i1], probs[ar, i2]
    ps = p1 + p2
    c1, c2 = p1 / ps, p2 / ps

    idx_list, coef_list = [], []
    for e in range(E):
        m1 = i1 == e
        m2 = i2 == e
        ide = np.nonzero(m1 | m2)[0]
        ce = np.where(m1[ide], c1[ide], c2[ide]).astype(np.float32)
        idx_list.append(ide)
        coef_list.append(ce)

    nmax = max(len(i) for i in idx_list)
    C = max(((nmax + P - 1) // P) * P, HEAD + 1536 + P)
    TT = C // P

    # ---- per-core input packing -------------------------------------------
    in_maps = []
    for e in range(E):
        ide, ce = idx_list[e], coef_list[e]
        ne = len(ide)

        xg = np.zeros((C, D), np.float32)
        xg[:ne] = xf[ide]
        xt_np = np.ascontiguousarray(xg.T).reshape(KT, P, C).astype(BF16_NP)
        xh_np = np.ascontiguousarray(xt_np[:, :, :HEAD])
        xtl_np = np.ascontiguousarray(xt_np[:, :, HEAD:])

        w1e = np.asarray(W1[e], np.float32)  # [H, D]
        w3e = np.asarray(W3[e], np.float32)  # [H, D]
        w2e = np.asarray(W2[e], np.float32)  # [D, H]
        # [m, h, k, d] -> [m, d, k, h] : packed[m][d, k*128+h] = W1[m*128+h, k*128+d]
        w1p_np = np.ascontiguousarray(
            w1e.reshape(MT, P, KT, P).transpose(0, 3, 2, 1)
        ).reshape(MT, P, KT * P).astype(BF16_NP)
        w3p_np = np.ascontiguousarray(
            w3e.reshape(MT, P, KT, P).transpose(0, 3, 2, 1)
        ).reshape(MT, P, KT * P).astype(BF16_NP)
        # W2T[h, d] tiles: [hk, h, d]
        w2p_np = np.ascontiguousarray(w2e.T).reshape(MT, P, D).astype(BF16_NP)

        cfe = np.zeros(C, np.float32)
        cfe[:ne] = ce
        cf_np = np.ascontiguousarray(cfe.reshape(TT, P).T)

        in_maps.append(
            {"xh": xh_np, "xtl": xtl_np, "w1p": w1p_np, "w3p": w3p_np,
             "w2p": w2p_np, "cf": cf_np}
        )

    # ---- build + run on 8 cores -------------------------------------------
    nc = build_kernel(C)
    res = None
    last_exc = None
    for attempt in range(3):
        try:
            res = run_bass_kernel_spmd(
                nc, in_maps, core_ids=list(range(E)),
                trace=TRACE and attempt == 0,
            )
            break
        except Exception as exc:  # transient device wedge / trace plumbing
            last_exc = exc
    if res is None:
        raise last_exc
    LAST_RESULTS = res

    # ---- combine ----------------------------------------------------------
    out = np.zeros((N, D), np.float32)
    for e in range(E):
        ide = idx_list[e]
        oe = res.results[e]["out"]  # [TT, 2, P, 512] bf16
        oe = oe.astype(np.float32).transpose(0, 2, 1, 3).reshape(C, D)
        out[ide] += oe[: len(ide)]

    return out.reshape(B, S, D)
